# revision 36
# baseline (speedup 1.0000x reference)
"""Distributed multi-head attention kernel for 8 TRN2 NeuronCores.

Reference computation:
    x:[2,2048,1024] -> qkv -> 16-head attention -> proj -> [2,2048,1024]

Sharding: tensor-parallel over heads (2 heads/core) for qkv + attention,
then an AllToAll switches to token sharding (512 tokens/core) for the
projection, so no AllReduce is needed and each core emits only its own
output shard.

Schedule (per core; PE computes out = lhsT.T @ rhs with contraction on
the partition axis). ACT must run one [128,1024] Exp per key-chunk
(~1.1us each, 128 chunks); the PE runs QK/PV plus all qkv production,
and under the observed power throttle (~1.2GHz effective PE clock) the
two engines are roughly balanced, so everything is pipelined:
  - x^T arrives token-block-major ([128, tb, c, 512] bf16); block 0/1
    are split across both DMA queues so span-0 q/k matmuls start right
    after the NEFF preamble; later x blocks and proj weights are DMA'd
    from inside the background stream so the sync queue stays in
    deadline order.
  - q/k computed d-major ([128 dims, 512 toks] tiles, N=512). V is also
    computed d-major (cheap N=512 matmuls, bias via tensor_scalar), then
    PE-transposed ([128,128] transpose-mode matmuls + DVE copy) into the
    per-chunk token-major [1|V] PV layout -- no DMA-xbar transposes
    (those serialize against x loads on the sync queue's semaphores).
  - attention: S^T = K.T @ Q per chunk; the two heads' QK matmuls run
    concurrently (row groups 0-63/64-127 via base_partition-derived
    tile_position), exp on ACT with the 1/8 scale folded in (no max
    subtraction: score std ~0.33), PV with lhsT=[1|V] so PSUM row 0
    accumulates the softmax denominator Z for free; the two heads' PV
    matmuls also overlap. QK runs one chunk ahead of PV.
  - all qkv/V production is yielded as background units pulled into the
    attention loop between QK(i+1) and PV(i), paced by an issue-deadline
    table (need[]): every producer instruction must be ISSUED before its
    first consumer or the Tile framework cannot order them.
  - per span: the u PSUM bank is evacuated to SBUF with one cheap DVE
    copy (so the next span's PV starts ~2us earlier), then Z row ->
    reciprocal (DVE), partition_broadcast (GPSIMD), normalize (DVE),
    DMA to the AllToAll staging buffer.
  - a tiny warm AllToAll early on absorbs the collective path's fixed
    start cost; the real AllToAll (1MB bf16) then feeds the
    token-sharded proj with full weights, bias via ACT Identity.
"""
import sys

sys.path.insert(0, "/opt/trn_rl_repo")

import numpy as np
import ml_dtypes

from concourse import bass, bacc, mybir, tile
from concourse.bass_utils import run_bass_kernel_spmd

BF16 = mybir.dt.bfloat16
F32 = mybir.dt.float32
I16 = mybir.dt.int16
AF = mybir.ActivationFunctionType
ALU = mybir.AluOpType

N_CORES = 8
B, N, C = 2, 2048, 1024
H = 16  # total heads
D = 64  # head dim
T = B * N  # 4096 flattened tokens
TPC = T // N_CORES  # tokens per core = 512
CCH = C // 128  # contraction chunks = 8
TB = T // 512  # token blocks = 8
SCALE = 1.0 / np.sqrt(D)  # 0.125
# Schraudolph bit-trick exp for head 1, bf16 variant: exp(s*SCALE) ~=
# bitcast_bf16(int16(TRICK_A*s + TRICK_B)). Scaled scores span ~+-2.2,
# far from overflow. Verified end-to-end in numpy: rel err 5.4e-3
# (gate 2e-2); halves the ACT-side exp work per chunk.
TRICK_A = (2.0 ** 7 / np.log(2.0)) * SCALE
TRICK_B = 127.0 * 128.0 - 366000.0 / 65536.0

import os as _os_mod

# PE warm-up matmuls while x DMA lands: HAM un-throttles (K=4/8 ->
# 8/8) after ~3.4us of sustained matmul activity, so burning junk
# matmuls from ~1us keeps the first real matmuls (~10us, once the
# critical DMA lands) at full clock instead of 1.2GHz.
JUNK_START = int(_os_mod.environ.get("KJUNK", "24"))
JUNK_A2A = 0  # PE keep-warm matmuls across the final AllToAll (net-negative, off)


def build_bass() -> bass.Bass:
    nc = bacc.Bacc(None, target_bir_lowering=False)

    # ---- DRAM parameters (per-core shards, host-prepared layouts) ----
    xt_d = nc.declare_dram_parameter("xt", [128, TB, CCH, 512], BF16, isOutput=False)
    # j-major (q/k/v-major) so the eager k tile's weights (j=1) land first
    wqkv_d = nc.declare_dram_parameter("wqkv", [128, 3, CCH, 128], BF16, isOutput=False)
    bqkv_d = nc.declare_dram_parameter("bqkv", [128, 3], F32, isOutput=False)
    ident_d = nc.declare_dram_parameter("ident", [128, 128], BF16, isOutput=False)
    pwt_d = nc.declare_dram_parameter("pwt", [128, CCH, C], BF16, isOutput=False)
    pb_d = nc.declare_dram_parameter("pb", [128, CCH], F32, isOutput=False)
    out_d = nc.declare_dram_parameter("out", [128, CCH, TPC], BF16, isOutput=True)
    import os
    DEBUG = os.environ.get("KDEBUG", "0") == "1"
    if DEBUG:
        qk_dump = nc.declare_dram_parameter("qk_dump", [128, 2, T], BF16, isOutput=True)
        vt_dump = nc.declare_dram_parameter("vt_dump", [128, T // 128, 2, 65], BF16, isOutput=True)

    with tile.TileContext(nc) as tc:
        with (
            tc.tile_pool(name="const", bufs=1) as const,
            tc.tile_pool(name="weights", bufs=1) as wpool,
            tc.tile_pool(name="acts", bufs=1) as apool,
            tc.tile_pool(name="dram", bufs=1, space="DRAM") as dram,
        ):
            # ---- resident SBUF tensors ----
            xt = wpool.tile([128, TB, CCH, 512], BF16)
            wqkv = wpool.tile([128, CCH, 384], BF16)
            pwt = wpool.tile([128, CCH, C], BF16)
            pb = const.tile([128, CCH], F32)
            bqkv = const.tile([128, 3], F32)
            ident = const.tile([128, 128], BF16)

            qk_sb = apool.tile([128, 2, T], BF16)  # j-major q/k
            # PV stationary: per t2-chunk, per head: [1 | V_h]
            vt = apool.tile([128, T // 128, 2, 65], BF16)
            a2a_sb = apool.tile([128, CCH, TPC], BF16)
            out_sb = apool.tile([128, CCH, TPC], BF16)
            jwarm = const.tile([128, 512], BF16)
            wsrc = const.tile([128, 32], F32)
            wdst = const.tile([128, 32], BF16)

            # token-striped AllToAll in 3 pieces: core c owns 256 tokens
            # of batch 0 (shipped after batch 0, fully overlapped), 128
            # tokens of batch 1 spans 0-1 (shipped mid-batch-1, also
            # overlapped), and 128 tokens of spans 2-3 (the only serial
            # collective, 256KB). The tail then runs 3/4 of the proj
            # while the last A2A flies.
            a2a_in0 = dram.tile([N_CORES * 128, TPC // 2], BF16)
            a2a_out0 = dram.tile([N_CORES * 128, TPC // 2], BF16)
            a2a_in1 = dram.tile([N_CORES * 128, TPC // 4], BF16)
            a2a_out1 = dram.tile([N_CORES * 128, TPC // 4], BF16)
            a2a_in1c = dram.tile([N_CORES * 128, TPC // 8], BF16)
            a2a_out1c = dram.tile([N_CORES * 128, TPC // 8], BF16)
            a2a_in2 = dram.tile([N_CORES * 128, TPC // 8], BF16)
            a2a_out2 = dram.tile([N_CORES * 128, TPC // 8], BF16)
            warm_in = dram.tile([N_CORES, 16], BF16)
            warm_out = dram.tile([N_CORES, 16], BF16)

            # ---- load inputs. sync queue: weights + most of x; scalar
            # queue: small consts + the other half of tb0/tb1 so span-0
            # inputs land fast. pwt/pb late (needed only at proj).
            # critical path first: the eager k0 tile's c=0 matmul needs
            # only wqkv[0:2] + xt[b0,c0:2] (~0.45MB), so those two land
            # before the bulk and the PE starts ~5us earlier.
            # strict priority order on ONE queue: the early phase is DMA-
            # bandwidth-bound (~0.2-0.4 MB/us), so a second queue's bulk
            # transfer steals bandwidth from the critical path. Eager
            # order of consumption: k0 (wqkv j=1 + xt b0), q0 (j=0),
            # v0 (j=2), then k1/v1 (xt b1).
            nc.sync.dma_start(out=wqkv[:, :, 128:256], in_=wqkv_d[:, 1])
            nc.sync.dma_start(out=xt[:, 0, 0:3, :], in_=xt_d[:, 0, 0:3, :])
            nc.sync.dma_start(out=wqkv[:, :, 0:128], in_=wqkv_d[:, 0])
            nc.sync.dma_start(out=xt[:, 0, 3:6, :], in_=xt_d[:, 0, 3:6, :])
            nc.sync.dma_start(out=wqkv[:, :, 256:384], in_=wqkv_d[:, 2])
            nc.sync.dma_start(out=xt[:, 0, 6:8, :], in_=xt_d[:, 0, 6:8, :])
            nc.scalar.dma_start(out=bqkv[:], in_=bqkv_d[:])
            nc.scalar.dma_start(out=ident[:], in_=ident_d[:])
            nc.sync.dma_start(out=xt[:, 1, 0:4, :], in_=xt_d[:, 1, 0:4, :])
            nc.sync.dma_start(out=xt[:, 1, 4:8, :], in_=xt_d[:, 1, 4:8, :])
            nc.sync.dma_start(out=xt[:, 2, :, :], in_=xt_d[:, 2, :, :])
            nc.sync.dma_start(out=xt[:, 3, :, :], in_=xt_d[:, 3, :, :])

            # preload the Exp table set so span 0 doesn't pay ~2.7us
            nc.vector.memset(wsrc[:], 0.0)
            nc.scalar.activation(wdst[:], wsrc[:], AF.Exp, scale=1.0)
            nc.vector.memset(jwarm[:], 0.25)
            nc.vector.memset(vt[:, :, :, 0:1], 1.0)  # Z ones columns

            # tiny dummy AllToAll: warms the collective path so the real
            # A2A's fixed start latency overlaps compute instead. Optional:
            # its completion semaphore gates later sync-queue DMA setup
            # (semaphore reuse), which can stall the V transposes.
            import os as _os
            if _os.environ.get("KWARM_A2A", "1") == "1":
                nc.gpsimd.collective_compute(
                    "AllToAll",
                    ALU.bypass,
                    replica_groups=[list(range(N_CORES))],
                    ins=[warm_in[:].opt()],
                    outs=[warm_out[:].opt()],
                )

            # ---- PE warm-up: HAM flips to 2.4GHz after ~3.4us of
            # sustained matmul activity; burn that in while x DMA lands.
            with tc.tile_pool(name="jw_psum", bufs=2, space="PSUM") as jwp:
                for _ in range(JUNK_START):
                    jt = jwp.tile([128, 512], F32, tag="jk", name="jk")
                    nc.tensor.matmul(
                        jt[:],
                        jwarm[:, 0:128],
                        jwarm[:],
                        start=True,
                        stop=True,
                        skip_group_check=True,
                    )

            # ---- background qkv/V production (pulled into PE slack) ----
            def producer(qp, vsp):
                def qk_tile(j, tb):
                    """One [128 dims, 512 toks] q (j=0) or k (j=1) tile."""
                    ps = qp.tile([128, 512], F32, tag="qkt", name="qkt")
                    for c in range(CCH):
                        nc.tensor.matmul(
                            ps[:],
                            wqkv[:, c, j * 128 : (j + 1) * 128],
                            xt[:, tb, c, :],
                            start=(c == 0),
                            stop=(c == CCH - 1),
                            skip_group_check=True,
                        )
                        yield
                    nc.vector.tensor_scalar_add(
                        qk_sb[:, j, tb * 512 : (tb + 1) * 512],
                        ps[:],
                        bqkv[:, j : j + 1],
                    )
                    yield

                def v_tile(tb):
                    """V d-major tile [128 vdims, 512 toks] (N=512 matmuls,
                    cheap), bias folded in d-major, then 4 PE transposes
                    into the per-chunk token-major [1|V] PV layout."""
                    ps = qp.tile([128, 512], F32, tag="qkt", name="vtl")
                    for c in range(CCH):
                        nc.tensor.matmul(
                            ps[:],
                            wqkv[:, c, 256:384],
                            xt[:, tb, c, :],
                            start=(c == 0),
                            stop=(c == CCH - 1),
                            skip_group_check=True,
                        )
                        yield
                    vs = vsp.tile([128, 512], BF16, tag="vst", name="vst")
                    nc.vector.tensor_scalar_add(vs[:], ps[:], bqkv[:, 2:3])
                    yield
                    for u in range(4):
                        ch = 4 * tb + u
                        # single-buffered: PSUM tags are bank-rounded
                        # and all 8 banks are allocated, so the transpose
                        # cannot be double-buffered
                        tt = qp.tile([128, 128], BF16, tag="ttl", name="ttl")
                        nc.tensor.transpose(
                            tt[:], vs[:, u * 128 : (u + 1) * 128], ident[:]
                        )
                        yield
                        nc.vector.tensor_copy(vt[:, ch, :, 1:65], tt[:, :])
                        yield

                def x_load(tb):
                    """Deferred input DMA issue, ordered into the sync
                    queue between the generator's transposes."""
                    if tb < TB:
                        nc.sync.dma_start(
                            out=xt[:, tb, :, :], in_=xt_d[:, tb, :, :]
                        )
                    elif tb == TB:
                        nc.sync.dma_start(out=pwt[:], in_=pwt_d[:])
                    else:
                        nc.sync.dma_start(out=pb[:], in_=pb_d[:])
                    yield

                # eager prefix: span-0 needs k0, q0 and the first V chunks
                yield from qk_tile(1, 0)
                yield from qk_tile(0, 0)
                yield from v_tile(0)
                # background, deadline order (see BG_ORDER)
                for kind, idx, _d, _n in BG_ORDER:
                    if kind == "k":
                        yield from qk_tile(1, idx)
                    elif kind == "q":
                        yield from qk_tile(0, idx)
                    elif kind == "x":
                        yield from x_load(idx)
                    else:
                        yield from v_tile(idx)
                while True:
                    yield

            QKU = CCH + 1  # yields per q/k tile
            VU = CCH + 1 + 8  # yields per v tile (8 mm, bias, 4x(tp+copy))
            EAGER = 2 * QKU + VU  # k0, q0, v0

            # Background units: (kind, idx, issue-deadline, yields).
            # deadline = global chunk index by whose END the unit must be
            # fully ISSUED -- program order must put every producer before
            # its first consumer or Tile cannot see the dependency.
            # k(tb): first read by qk(4*(tb%4)) of its batch, one chunk
            # early. q(tb): first read by qk(16*s). v(tb): chunks
            # 4tb..4tb+3, first read by PV(4*(tb%4)). x(tb): deferred
            # input DMA issue; x8/x9 = pwt/pb.
            BG_ORDER = [
                ("k", 1, 1, QKU), ("v", 1, 2, VU), ("k", 2, 5, QKU),
                ("v", 2, 6, VU), ("k", 3, 9, QKU), ("v", 3, 10, VU),
                ("q", 1, 13, QKU), ("q", 2, 20, QKU), ("x", 4, 22, 1),
                ("x", 5, 26, 1), ("q", 3, 28, QKU), ("x", 6, 32, 1),
                ("x", 7, 34, 1), ("q", 4, 38, QKU), ("k", 4, 42, QKU),
                ("x", 8, 44, 1), ("x", 9, 45, 1), ("v", 4, 58, VU),
                ("k", 5, 65, QKU), ("v", 5, 66, VU), ("k", 6, 69, QKU),
                ("v", 6, 70, VU), ("k", 7, 73, QKU), ("v", 7, 74, VU),
                ("q", 5, 77, QKU), ("q", 6, 93, QKU), ("q", 7, 109, QKU),
            ]
            TOTAL_BG = sum(n for _, _, _, n in BG_ORDER)
            # need[g] = total bg yields that must be issued by the end of
            # global chunk g
            need = [0] * 128
            for _kind, _idx, _d, _n in BG_ORDER:
                for _g in range(max(_d, 0), 128):
                    need[_g] += _n

            def attention_batch(b, sp, up, np_pool, ptp, ptp1, zp, gen, pulled,
                                mid_hook=None):
                """Attention for batch b; flat chunk loop, QK one chunk
                ahead of PV across span boundaries."""
                nchunks = N // 128
                nspans = N // 512
                total = nspans * nchunks
                s_tiles = {}
                u_tiles = {}

                def qk(i):
                    span, ch = divmod(i, nchunks)
                    t1 = b * N + span * 512
                    t2 = b * N + ch * 128
                    s = sp.tile([128, 1024], F32, tag="s", name="s")
                    s_tiles[i] = s
                    nc.tensor.matmul(
                        s[:, 0:512],
                        qk_sb[0:64, 1, t2 : t2 + 128],
                        qk_sb[0:64, 0, t1 : t1 + 512],
                        start=True,
                        stop=True,
                    )
                    nc.tensor.matmul(
                        s[:, 512:1024],
                        qk_sb[64:128, 1, t2 : t2 + 128],
                        qk_sb[64:128, 0, t1 : t1 + 512],
                        start=True,
                        stop=True,
                    )

                def finalize_span(span):
                    u0, u1 = u_tiles.pop(span)
                    last = b == 1 and span == nspans - 1
                    srcs = []
                    if not last:
                        # evacuate both u PSUM banks first (cheap copies)
                        # so the next span's PV can start ~2us earlier;
                        # the normalize chain then runs off-critical
                        for h, u in ((0, u0), (1, u1)):
                            un = np_pool.tile(
                                [65, 512], F32, tag=f"un{h}", name="un"
                            )
                            nc.vector.tensor_copy(un[0:65, :], u[0:65, :])
                            srcs.append(un)
                    else:
                        # final span: nothing waits on the banks, and the
                        # copies would only delay the AllToAll gate
                        srcs = [u0, u1]
                    zbs = []
                    for h in (0, 1):
                        zinv = zp.tile([1, 512], F32, tag=f"z{h}", name="zi")
                        nc.vector.reciprocal_approx_fast(
                            zinv[0:1, :], srcs[h][0:1, :]
                        )
                        zb = zp.tile([65, 512], F32, tag=f"zb{h}", name="zb")
                        nc.gpsimd.partition_broadcast(
                            zb[0:65, :], zinv[0:1, :], channels=65
                        )
                        zbs.append(zb)
                    for h in (0, 1):
                        nrm = np_pool.tile([65, 512], BF16, tag=f"n{h}", name="nr")
                        nc.vector.tensor_tensor(
                            nrm[0:65, :],
                            srcs[h][0:65, :],
                            zbs[h][0:65, :],
                            op=ALU.mult,
                        )
                        # stripe into this span's collective buffer.
                        # batch 0: 256-col blocks, cores 2s/2s+1.
                        # batch 1: 128-col blocks, cores 4s'..4s'+3 of
                        # a2a_in1 (spans 0-1) / a2a_in2 (spans 2-3).
                        # mid-attention spans stage on the sync queue (the
                        # scalar queue is busy with exps); the last span
                        # splits across sync+scalar to halve the ~0.55us
                        # per-DMA issue latency ahead of the A2A trigger.
                        # ONE strided DMA per head covers all destination
                        # blocks: dst viewed [row-in-block, block, col],
                        # src free dim split [q-block, col] to match.
                        if b == 0:
                            dst = a2a_in0[:].rearrange(
                                "(blk r) c -> r blk c", blk=N_CORES
                            )[h * 64 : (h + 1) * 64, 2 * span : 2 * span + 2, :]
                            src = nrm[1:65, :].rearrange("r (q c) -> r q c", q=2)
                            nc.sync.dma_start(out=dst, in_=src)
                        elif span < 2:
                            # spans 0-1: 128-col blocks of a2a_in1
                            dst = a2a_in1[:].rearrange(
                                "(blk r) c -> r blk c", blk=N_CORES
                            )[h * 64 : (h + 1) * 64, 4 * span : 4 * span + 4, :]
                            src = nrm[1:65, :].rearrange("r (q c) -> r q c", q=4)
                            nc.sync.dma_start(out=dst, in_=src)
                        else:
                            # spans 2/3: own 64-col-block collectives.
                            # span 2 stages on gpsimd (SWDGE): the sync
                            # queue is clogged by the A2A#1b-gated a2a_sb
                            # loads right then. span 3 (the final trigger
                            # gate): one head per idle queue.
                            a2a_in = a2a_in1c if span == 2 else a2a_in2
                            dst = a2a_in[:].rearrange(
                                "(blk r) c -> r blk c", blk=N_CORES
                            )[h * 64 : (h + 1) * 64, :, :]
                            src = nrm[1:65, :].rearrange(
                                "r (q c) -> r q c", q=N_CORES
                            )
                            if last:
                                eng = nc.sync if h == 0 else nc.scalar
                            else:
                                eng = nc.gpsimd
                            eng.dma_start(out=dst, in_=src)

                def pv(j):
                    """exp + PV for chunk j. Emitted one iter AFTER
                    qk(j+1) and one iter BEFORE qk(j+2): the PE order per
                    iter is [prods, PV(j), QK(j+2)], so every exp's QK
                    input finished a full iteration earlier and the ACT
                    exp stream runs back-to-back (it is the ~1.17us/chunk
                    floor); the PE burst (prods+PV+QK ~1.2us) hides under
                    it. qk(j+2) reuses s(j)'s PSUM bank (ring of 2), whose
                    WAR on exp(j) resolves just before PV(j) anyway."""
                    span, ch = divmod(j, nchunks)
                    if ch == 0:
                        u_tiles[span] = (
                            up.tile([128, 512], F32, tag="u0", name="u0"),
                            up.tile([128, 512], F32, tag="u1", name="u1"),
                        )
                    u0, u1 = u_tiles[span]
                    s = s_tiles.pop(j)
                    pt = ptp.tile([128, 1024], BF16, tag="pt", name="pt")
                    gch = (b * N + ch * 128) // 128
                    # Both heads' exp on ACT. (A DVE bit-trick exp for
                    # head 1 was tried and reverted: every DVE op pays a
                    # pipeline DRAIN roughly equal to its duration, so the
                    # trick chain (~1.3us) gated PV worse than the 1.08us
                    # ACT exp it replaced.)
                    nc.scalar.activation(pt[:], s[:], AF.Exp, scale=SCALE)
                    nc.tensor.matmul(
                        u0[0:65, :],
                        vt[:, gch, 0, :],
                        pt[:, 0:512],
                        start=(ch == 0),
                        stop=(ch == nchunks - 1),
                    )
                    nc.tensor.matmul(
                        u1[0:65, :],
                        vt[:, gch, 1, :],
                        pt[:, 512:1024],
                        start=(ch == 0),
                        stop=(ch == nchunks - 1),
                    )
                    if ch == nchunks - 1:
                        finalize_span(span)

                qk(0)
                for i in range(total):
                    while pulled[0] < need[b * 64 + i]:
                        next(gen)
                        pulled[0] += 1
                    if i >= 1:
                        pv(i - 1)
                        if mid_hook is not None and (i - 1) in mid_hook:
                            mid_hook[i - 1]()
                    if i + 1 < total:
                        qk(i + 1)
                pv(total - 1)

            # ---- attention both batches; qkv/V production in the slack ----
            with (
                tc.tile_pool(name="norm", bufs=4) as np_pool,
                tc.tile_pool(name="pt", bufs=6) as ptp,
                tc.tile_pool(name="pti", bufs=4) as ptp1,
                tc.tile_pool(name="zrow", bufs=4) as zp,
                tc.tile_pool(name="vstage", bufs=2) as vsp,
                tc.tile_pool(name="s_psum", bufs=2, space="PSUM") as sp,
                tc.tile_pool(name="u_psum", bufs=1, space="PSUM") as up,
                tc.tile_pool(name="qk_psum", bufs=1, space="PSUM") as qp,
            ):
                gen = producer(qp, vsp)
                for _ in range(EAGER):
                    next(gen)
                pulled = [0]
                attention_batch(0, sp, up, np_pool, ptp, ptp1, zp, gen, pulled)

                # ---- batch 0's AllToAll, fully overlapped with batch 1's
                # attention; the a2a_sb loads for the b0 half also drain
                # here (sync queue is idle mid-attention).
                nc.gpsimd.collective_compute(
                    "AllToAll",
                    ALU.bypass,
                    replica_groups=[list(range(N_CORES))],
                    ins=[a2a_in0[:].opt()],
                    outs=[a2a_out0[:].opt()],
                )
                nc.sync.dma_start(
                    out=a2a_sb[:, :, 0 : TPC // 2],
                    in_=a2a_out0[:].rearrange("(g r) c -> r g c", g=N_CORES),
                )

                def b1_mid():
                    # batch-1 spans 0-1 staged: ship them now (overlaps
                    # the rest of batch 1), load into a2a_sb cols 256:384
                    nc.gpsimd.collective_compute(
                        "AllToAll",
                        ALU.bypass,
                        replica_groups=[list(range(N_CORES))],
                        ins=[a2a_in1[:].opt()],
                        outs=[a2a_out1[:].opt()],
                    )
                    nc.sync.dma_start(
                        out=a2a_sb[:, :, TPC // 2 : 3 * TPC // 4],
                        in_=a2a_out1[:].rearrange("(g r) c -> r g c", g=N_CORES),
                    )

                def b1_mid2():
                    # batch-1 span 2 staged: ship it too (overlaps span 3)
                    nc.gpsimd.collective_compute(
                        "AllToAll",
                        ALU.bypass,
                        replica_groups=[list(range(N_CORES))],
                        ins=[a2a_in1c[:].opt()],
                        outs=[a2a_out1c[:].opt()],
                    )
                    nc.sync.dma_start(
                        out=a2a_sb[:, :, 3 * TPC // 4 : 7 * TPC // 8],
                        in_=a2a_out1c[:].rearrange("(g r) c -> r g c", g=N_CORES),
                    )

                attention_batch(1, sp, up, np_pool, ptp, ptp1, zp, gen, pulled,
                                mid_hook={31: b1_mid, 47: b1_mid2})
                while pulled[0] < TOTAL_BG:
                    next(gen)
                    pulled[0] += 1

            if DEBUG:
                nc.sync.dma_start(out=qk_dump[:], in_=qk_sb[:])
                nc.sync.dma_start(out=vt_dump[:], in_=vt[:])

            # ---- batch-1 span 3: the only serial collective (128KB) ----
            nc.gpsimd.collective_compute(
                "AllToAll",
                ALU.bypass,
                replica_groups=[list(range(N_CORES))],
                ins=[a2a_in2[:].opt()],
                outs=[a2a_out2[:].opt()],
            )

            # ---- proj (full weights, my striped tokens), in two pieces:
            # cols 0:448 arrived with the overlapped collectives, so 7/8
            # of the proj fills the PE while the last A2A flies; only
            # cols 448:512 are serial after it.
            q3 = 7 * TPC // 8
            with (
                tc.tile_pool(name="proj_psum", bufs=4, space="PSUM") as pp,
            ):
                def proj_piece(lo, hi, tag):
                    for o in range(CCH):
                        ps = pp.tile([128, hi - lo], F32, tag=tag, name=tag)
                        for g in range(CCH):
                            nc.tensor.matmul(
                                ps[:],
                                pwt[:, g, o * 128 : (o + 1) * 128],
                                a2a_sb[:, g, lo:hi],
                                start=(g == 0),
                                stop=(g == CCH - 1),
                            )
                        nc.scalar.activation(
                            out_sb[:, o, lo:hi], ps[:], AF.Identity,
                            bias=pb[:, o : o + 1],
                        )
                        eng = nc.sync if o % 2 == 0 else nc.scalar
                        eng.dma_start(
                            out=out_d[:, o, lo:hi], in_=out_sb[:, o, lo:hi]
                        )

                proj_piece(0, q3, "pps0")
                nc.sync.dma_start(
                    out=a2a_sb[:, :, q3:TPC],
                    in_=a2a_out2[:].rearrange("(g r) c -> r g c", g=N_CORES),
                )
                proj_piece(q3, TPC, "pps1")

    nc.compile()
    return nc


def shard_inputs(x, qkv_w, qkv_b, proj_w, proj_b):
    """Host-side sharding + layout prep. Returns in_maps for 8 cores."""
    bf = ml_dtypes.bfloat16
    x2 = np.ascontiguousarray(x.reshape(T, C).T)  # [C, T]
    xt = np.ascontiguousarray(
        x2.reshape(CCH, 128, TB, 512).transpose(1, 2, 0, 3)
    ).astype(bf)  # [128, tb, c, 512]
    pwt_f = np.ascontiguousarray(proj_w.T)  # [j, o]
    pwt = np.ascontiguousarray(
        pwt_f.reshape(CCH, 128, C).transpose(1, 0, 2)
    ).astype(bf)
    pb = np.ascontiguousarray(proj_b.reshape(CCH, 128).T).astype(np.float32)
    ident = np.eye(128, dtype=ml_dtypes.bfloat16)

    in_maps = []
    for c in range(N_CORES):
        rows = lambda base: slice(base + 128 * c, base + 128 * (c + 1))
        wq = qkv_w[rows(0)]
        wk = qkv_w[rows(C)]
        wv = qkv_w[rows(2 * C)]
        wqkv = np.concatenate([wq, wk, wv], axis=0).T  # [C, 384]
        # j-major DRAM layout [128, 3(j), CCH, 128]
        wqkv = np.ascontiguousarray(
            wqkv.reshape(CCH, 128, 3, 128).transpose(1, 2, 0, 3)
        ).astype(bf)
        bqkv = np.stack(
            [qkv_b[rows(0)], qkv_b[rows(C)], qkv_b[rows(2 * C)]], axis=1
        ).astype(np.float32)
        in_maps.append(
            {
                "xt": xt, "wqkv": wqkv, "bqkv": bqkv, "ident": ident,
                "pwt": pwt, "pb": pb,
            }
        )
    return in_maps


_CACHED_NC = None


def kernel(x, qkv_w, qkv_b, proj_w, proj_b, _trace=False, _tmpdir=None):
    global _CACHED_NC
    x = np.asarray(x, dtype=np.float32)
    qkv_w = np.asarray(qkv_w, dtype=np.float32)
    qkv_b = np.asarray(qkv_b, dtype=np.float32)
    proj_w = np.asarray(proj_w, dtype=np.float32)
    proj_b = np.asarray(proj_b, dtype=np.float32)

    in_maps = shard_inputs(x, qkv_w, qkv_b, proj_w, proj_b)
    if _CACHED_NC is None:
        _CACHED_NC = build_bass()
    res = run_bass_kernel_spmd(
        _CACHED_NC,
        in_maps,
        core_ids=list(range(N_CORES)),
        trace=_trace,
        tmpdir=_tmpdir,
    )
    # out per core: [128, CCH, TPC] bf16; cols 0:256 = batch-0 tokens
    # [256c, 256c+256); cols 256:384 = batch-1 tokens [128c, 128c+128)
    # (spans 0-1); cols 384:448 = b1 span-2 tokens [1024+64c, +64);
    # cols 448:512 = b1 span-3 tokens [1536+64c, +64)
    out = np.empty((B, N, C), dtype=np.float32)
    for c in range(N_CORES):
        o = np.asarray(res.results[c]["out"]).astype(np.float32)
        ot = o.transpose(1, 0, 2).reshape(C, TPC)  # [1024 o, 512 t]
        out[0, c * 256 : (c + 1) * 256, :] = ot[:, 0:256].T
        out[1, c * 128 : (c + 1) * 128, :] = ot[:, 256:384].T
        out[1, 1024 + c * 64 : 1024 + (c + 1) * 64, :] = ot[:, 384:448].T
        out[1, 1536 + c * 64 : 1536 + (c + 1) * 64, :] = ot[:, 448:512].T
    if _trace:
        return out, res
    return out



# revision 37
# speedup vs baseline: 1.0012x; 1.0012x over previous
"""Distributed multi-head attention kernel for 8 TRN2 NeuronCores.

Reference computation:
    x:[2,2048,1024] -> qkv -> 16-head attention -> proj -> [2,2048,1024]

Sharding: tensor-parallel over heads (2 heads/core) for qkv + attention,
then an AllToAll switches to token sharding (512 tokens/core) for the
projection, so no AllReduce is needed and each core emits only its own
output shard.

Schedule (per core; PE computes out = lhsT.T @ rhs with contraction on
the partition axis). ACT must run one [128,1024] Exp per key-chunk
(~1.1us each, 128 chunks); the PE runs QK/PV plus all qkv production,
and under the observed power throttle (~1.2GHz effective PE clock) the
two engines are roughly balanced, so everything is pipelined:
  - x^T arrives token-block-major ([128, tb, c, 512] bf16); block 0/1
    are split across both DMA queues so span-0 q/k matmuls start right
    after the NEFF preamble; later x blocks and proj weights are DMA'd
    from inside the background stream so the sync queue stays in
    deadline order.
  - q/k computed d-major ([128 dims, 512 toks] tiles, N=512). V is also
    computed d-major (cheap N=512 matmuls, bias via tensor_scalar), then
    PE-transposed ([128,128] transpose-mode matmuls + DVE copy) into the
    per-chunk token-major [1|V] PV layout -- no DMA-xbar transposes
    (those serialize against x loads on the sync queue's semaphores).
  - attention: S^T = K.T @ Q per chunk; the two heads' QK matmuls run
    concurrently (row groups 0-63/64-127 via base_partition-derived
    tile_position), exp on ACT with the 1/8 scale folded in (no max
    subtraction: score std ~0.33), PV with lhsT=[1|V] so PSUM row 0
    accumulates the softmax denominator Z for free; the two heads' PV
    matmuls also overlap. QK runs one chunk ahead of PV.
  - all qkv/V production is yielded as background units pulled into the
    attention loop between QK(i+1) and PV(i), paced by an issue-deadline
    table (need[]): every producer instruction must be ISSUED before its
    first consumer or the Tile framework cannot order them.
  - per span: the u PSUM bank is evacuated to SBUF with one cheap DVE
    copy (so the next span's PV starts ~2us earlier), then Z row ->
    reciprocal (DVE), partition_broadcast (GPSIMD), normalize (DVE),
    DMA to the AllToAll staging buffer.
  - a tiny warm AllToAll early on absorbs the collective path's fixed
    start cost; the real AllToAll (1MB bf16) then feeds the
    token-sharded proj with full weights, bias via ACT Identity.
"""
import sys

sys.path.insert(0, "/opt/trn_rl_repo")

import numpy as np
import ml_dtypes

from concourse import bass, bacc, mybir, tile
from concourse.bass_utils import run_bass_kernel_spmd

BF16 = mybir.dt.bfloat16
F32 = mybir.dt.float32
I16 = mybir.dt.int16
AF = mybir.ActivationFunctionType
ALU = mybir.AluOpType

N_CORES = 8
B, N, C = 2, 2048, 1024
H = 16  # total heads
D = 64  # head dim
T = B * N  # 4096 flattened tokens
TPC = T // N_CORES  # tokens per core = 512
CCH = C // 128  # contraction chunks = 8
TB = T // 512  # token blocks = 8
SCALE = 1.0 / np.sqrt(D)  # 0.125
# Schraudolph bit-trick exp for head 1, bf16 variant: exp(s*SCALE) ~=
# bitcast_bf16(int16(TRICK_A*s + TRICK_B)). Scaled scores span ~+-2.2,
# far from overflow. Verified end-to-end in numpy: rel err 5.4e-3
# (gate 2e-2); halves the ACT-side exp work per chunk.
TRICK_A = (2.0 ** 7 / np.log(2.0)) * SCALE
TRICK_B = 127.0 * 128.0 - 366000.0 / 65536.0

import os as _os_mod

# PE warm-up matmuls while x DMA lands: HAM un-throttles (K=4/8 ->
# 8/8) after ~3.4us of sustained matmul activity, so burning junk
# matmuls from ~1us keeps the first real matmuls (~10us, once the
# critical DMA lands) at full clock instead of 1.2GHz.
JUNK_START = int(_os_mod.environ.get("KJUNK", "24"))
JUNK_A2A = 0  # PE keep-warm matmuls across the final AllToAll (net-negative, off)


def build_bass() -> bass.Bass:
    nc = bacc.Bacc(None, target_bir_lowering=False)

    # ---- DRAM parameters (per-core shards, host-prepared layouts) ----
    xt_d = nc.declare_dram_parameter("xt", [128, TB, CCH, 512], BF16, isOutput=False)
    # j-major (q/k/v-major) so the eager k tile's weights (j=1) land first
    wqkv_d = nc.declare_dram_parameter("wqkv", [128, 3, CCH, 128], BF16, isOutput=False)
    bqkv_d = nc.declare_dram_parameter("bqkv", [128, 3], F32, isOutput=False)
    ident_d = nc.declare_dram_parameter("ident", [128, 128], BF16, isOutput=False)
    pwt_d = nc.declare_dram_parameter("pwt", [128, CCH, C], BF16, isOutput=False)
    pb_d = nc.declare_dram_parameter("pb", [128, CCH], F32, isOutput=False)
    out_d = nc.declare_dram_parameter("out", [128, CCH, TPC], BF16, isOutput=True)
    import os
    DEBUG = os.environ.get("KDEBUG", "0") == "1"
    if DEBUG:
        qk_dump = nc.declare_dram_parameter("qk_dump", [128, 2, T], BF16, isOutput=True)
        vt_dump = nc.declare_dram_parameter("vt_dump", [128, T // 128, 2, 65], BF16, isOutput=True)

    with tile.TileContext(nc) as tc:
        with (
            tc.tile_pool(name="const", bufs=1) as const,
            tc.tile_pool(name="weights", bufs=1) as wpool,
            tc.tile_pool(name="acts", bufs=1) as apool,
            tc.tile_pool(name="dram", bufs=1, space="DRAM") as dram,
        ):
            # ---- resident SBUF tensors ----
            xt = wpool.tile([128, TB, CCH, 512], BF16)
            wqkv = wpool.tile([128, CCH, 384], BF16)
            pwt = wpool.tile([128, CCH, C], BF16)
            pb = const.tile([128, CCH], F32)
            bqkv = const.tile([128, 3], F32)
            ident = const.tile([128, 128], BF16)

            qk_sb = apool.tile([128, 2, T], BF16)  # j-major q/k
            # PV stationary: per t2-chunk, per head: [1 | V_h]
            vt = apool.tile([128, T // 128, 2, 65], BF16)
            a2a_sb = apool.tile([128, CCH, TPC], BF16)
            out_sb = apool.tile([128, CCH, TPC], BF16)
            jwarm = const.tile([128, 512], BF16)
            wsrc = const.tile([128, 32], F32)
            wdst = const.tile([128, 32], BF16)

            # token-striped AllToAll in 3 pieces: core c owns 256 tokens
            # of batch 0 (shipped after batch 0, fully overlapped), 128
            # tokens of batch 1 spans 0-1 (shipped mid-batch-1, also
            # overlapped), and 128 tokens of spans 2-3 (the only serial
            # collective, 256KB). The tail then runs 3/4 of the proj
            # while the last A2A flies.
            a2a_in0 = dram.tile([N_CORES * 128, TPC // 2], BF16)
            a2a_out0 = dram.tile([N_CORES * 128, TPC // 2], BF16)
            a2a_in1 = dram.tile([N_CORES * 128, TPC // 4], BF16)
            a2a_out1 = dram.tile([N_CORES * 128, TPC // 4], BF16)
            a2a_in1c = dram.tile([N_CORES * 128, TPC // 8], BF16)
            a2a_out1c = dram.tile([N_CORES * 128, TPC // 8], BF16)
            a2a_in2 = dram.tile([N_CORES * 128, TPC // 8], BF16)
            a2a_out2 = dram.tile([N_CORES * 128, TPC // 8], BF16)
            warm_in = dram.tile([N_CORES, 16], BF16)
            warm_out = dram.tile([N_CORES, 16], BF16)

            # ---- load inputs. sync queue: weights + most of x; scalar
            # queue: small consts + the other half of tb0/tb1 so span-0
            # inputs land fast. pwt/pb late (needed only at proj).
            # critical path first: the eager k0 tile's c=0 matmul needs
            # only wqkv[0:2] + xt[b0,c0:2] (~0.45MB), so those two land
            # before the bulk and the PE starts ~5us earlier.
            # strict priority order on ONE queue: the early phase is DMA-
            # bandwidth-bound (~0.2-0.4 MB/us), so a second queue's bulk
            # transfer steals bandwidth from the critical path. Eager
            # order of consumption: k0 (wqkv j=1 + xt b0), q0 (j=0),
            # v0 (j=2), then k1/v1 (xt b1).
            nc.sync.dma_start(out=wqkv[:, :, 128:256], in_=wqkv_d[:, 1])
            nc.sync.dma_start(out=xt[:, 0, 0:3, :], in_=xt_d[:, 0, 0:3, :])
            nc.sync.dma_start(out=wqkv[:, :, 0:128], in_=wqkv_d[:, 0])
            nc.sync.dma_start(out=xt[:, 0, 3:6, :], in_=xt_d[:, 0, 3:6, :])
            nc.sync.dma_start(out=wqkv[:, :, 256:384], in_=wqkv_d[:, 2])
            nc.sync.dma_start(out=xt[:, 0, 6:8, :], in_=xt_d[:, 0, 6:8, :])
            nc.scalar.dma_start(out=bqkv[:], in_=bqkv_d[:])
            nc.scalar.dma_start(out=ident[:], in_=ident_d[:])
            nc.sync.dma_start(out=xt[:, 1, 0:4, :], in_=xt_d[:, 1, 0:4, :])
            nc.sync.dma_start(out=xt[:, 1, 4:8, :], in_=xt_d[:, 1, 4:8, :])
            nc.sync.dma_start(out=xt[:, 2, :, :], in_=xt_d[:, 2, :, :])
            nc.sync.dma_start(out=xt[:, 3, :, :], in_=xt_d[:, 3, :, :])

            # preload the Exp table set so span 0 doesn't pay ~2.7us
            nc.vector.memset(wsrc[:], 0.0)
            nc.scalar.activation(wdst[:], wsrc[:], AF.Exp, scale=1.0)
            nc.vector.memset(jwarm[:], 0.25)
            nc.vector.memset(vt[:, :, :, 0:1], 1.0)  # Z ones columns

            # tiny dummy AllToAll: warms the collective path so the real
            # A2A's fixed start latency overlaps compute instead. Optional:
            # its completion semaphore gates later sync-queue DMA setup
            # (semaphore reuse), which can stall the V transposes.
            import os as _os
            if _os.environ.get("KWARM_A2A", "1") == "1":
                nc.gpsimd.collective_compute(
                    "AllToAll",
                    ALU.bypass,
                    replica_groups=[list(range(N_CORES))],
                    ins=[warm_in[:].opt()],
                    outs=[warm_out[:].opt()],
                )

            # ---- PE warm-up: HAM flips to 2.4GHz after ~3.4us of
            # sustained matmul activity; burn that in while x DMA lands.
            with tc.tile_pool(name="jw_psum", bufs=2, space="PSUM") as jwp:
                for _ in range(JUNK_START):
                    jt = jwp.tile([128, 512], F32, tag="jk", name="jk")
                    nc.tensor.matmul(
                        jt[:],
                        jwarm[:, 0:128],
                        jwarm[:],
                        start=True,
                        stop=True,
                        skip_group_check=True,
                    )

            # ---- background qkv/V production (pulled into PE slack) ----
            def producer(qp, vsp):
                def qk_tile(j, tb):
                    """One [128 dims, 512 toks] q (j=0) or k (j=1) tile."""
                    ps = qp.tile([128, 512], F32, tag="qkt", name="qkt")
                    for c in range(CCH):
                        nc.tensor.matmul(
                            ps[:],
                            wqkv[:, c, j * 128 : (j + 1) * 128],
                            xt[:, tb, c, :],
                            start=(c == 0),
                            stop=(c == CCH - 1),
                            skip_group_check=True,
                        )
                        yield
                    nc.vector.tensor_scalar_add(
                        qk_sb[:, j, tb * 512 : (tb + 1) * 512],
                        ps[:],
                        bqkv[:, j : j + 1],
                    )
                    yield

                def v_tile(tb):
                    """V d-major tile [128 vdims, 512 toks] (N=512 matmuls,
                    cheap), bias folded in d-major, then 4 PE transposes
                    into the per-chunk token-major [1|V] PV layout."""
                    ps = qp.tile([128, 512], F32, tag="qkt", name="vtl")
                    for c in range(CCH):
                        nc.tensor.matmul(
                            ps[:],
                            wqkv[:, c, 256:384],
                            xt[:, tb, c, :],
                            start=(c == 0),
                            stop=(c == CCH - 1),
                            skip_group_check=True,
                        )
                        yield
                    vs = vsp.tile([128, 512], BF16, tag="vst", name="vst")
                    nc.vector.tensor_scalar_add(vs[:], ps[:], bqkv[:, 2:3])
                    yield
                    for u in range(4):
                        ch = 4 * tb + u
                        # single-buffered: PSUM tags are bank-rounded
                        # and all 8 banks are allocated, so the transpose
                        # cannot be double-buffered
                        tt = qp.tile([128, 128], BF16, tag="ttl", name="ttl")
                        nc.tensor.transpose(
                            tt[:], vs[:, u * 128 : (u + 1) * 128], ident[:]
                        )
                        yield
                        nc.vector.tensor_copy(vt[:, ch, :, 1:65], tt[:, :])
                        yield

                def x_load(tb):
                    """Deferred input DMA issue, ordered into the sync
                    queue between the generator's transposes."""
                    if tb < TB:
                        nc.sync.dma_start(
                            out=xt[:, tb, :, :], in_=xt_d[:, tb, :, :]
                        )
                    elif tb == TB:
                        nc.sync.dma_start(out=pwt[:], in_=pwt_d[:])
                    else:
                        nc.sync.dma_start(out=pb[:], in_=pb_d[:])
                    yield

                # eager prefix: span-0 needs k0, q0 and the first V chunks
                yield from qk_tile(1, 0)
                yield from qk_tile(0, 0)
                yield from v_tile(0)
                # background, deadline order (see BG_ORDER)
                for kind, idx, _d, _n in BG_ORDER:
                    if kind == "k":
                        yield from qk_tile(1, idx)
                    elif kind == "q":
                        yield from qk_tile(0, idx)
                    elif kind == "x":
                        yield from x_load(idx)
                    else:
                        yield from v_tile(idx)
                while True:
                    yield

            QKU = CCH + 1  # yields per q/k tile
            VU = CCH + 1 + 8  # yields per v tile (8 mm, bias, 4x(tp+copy))
            EAGER = 2 * QKU + VU  # k0, q0, v0

            # Background units: (kind, idx, issue-deadline, yields).
            # deadline = global chunk index by whose END the unit must be
            # fully ISSUED -- program order must put every producer before
            # its first consumer or Tile cannot see the dependency.
            # k(tb): first read by qk(4*(tb%4)) of its batch, one chunk
            # early. q(tb): first read by qk(16*s). v(tb): chunks
            # 4tb..4tb+3, first read by PV(4*(tb%4)). x(tb): deferred
            # input DMA issue; x8/x9 = pwt/pb.
            BG_ORDER = [
                ("k", 1, 1, QKU), ("v", 1, 2, VU), ("k", 2, 5, QKU),
                ("v", 2, 6, VU), ("k", 3, 9, QKU), ("v", 3, 10, VU),
                ("q", 1, 13, QKU), ("q", 2, 20, QKU), ("x", 4, 22, 1),
                ("x", 5, 26, 1), ("q", 3, 28, QKU), ("x", 6, 32, 1),
                ("x", 7, 34, 1), ("q", 4, 38, QKU), ("k", 4, 42, QKU),
                ("x", 8, 44, 1), ("x", 9, 45, 1), ("v", 4, 58, VU),
                ("k", 5, 65, QKU), ("v", 5, 66, VU), ("k", 6, 69, QKU),
                ("v", 6, 70, VU), ("k", 7, 73, QKU), ("v", 7, 74, VU),
                ("q", 5, 77, QKU), ("q", 6, 93, QKU), ("q", 7, 109, QKU),
            ]
            TOTAL_BG = sum(n for _, _, _, n in BG_ORDER)
            # need[g] = total bg yields that must be issued by the end of
            # global chunk g
            need = [0] * 128
            for _kind, _idx, _d, _n in BG_ORDER:
                for _g in range(max(_d, 0), 128):
                    need[_g] += _n

            def attention_batch(b, sp, up, np_pool, ptp, ptp1, zp, gen, pulled,
                                mid_hook=None):
                """Attention for batch b; flat chunk loop, QK one chunk
                ahead of PV across span boundaries."""
                nchunks = N // 128
                nspans = N // 512
                total = nspans * nchunks
                s_tiles = {}
                u_tiles = {}

                def qk(i):
                    span, ch = divmod(i, nchunks)
                    t1 = b * N + span * 512
                    t2 = b * N + ch * 128
                    s = sp.tile([128, 1024], F32, tag="s", name="s")
                    s_tiles[i] = s
                    nc.tensor.matmul(
                        s[:, 0:512],
                        qk_sb[0:64, 1, t2 : t2 + 128],
                        qk_sb[0:64, 0, t1 : t1 + 512],
                        start=True,
                        stop=True,
                    )
                    nc.tensor.matmul(
                        s[:, 512:1024],
                        qk_sb[64:128, 1, t2 : t2 + 128],
                        qk_sb[64:128, 0, t1 : t1 + 512],
                        start=True,
                        stop=True,
                    )

                def finalize_span(span):
                    u0, u1 = u_tiles.pop(span)
                    last = b == 1 and span == nspans - 1
                    srcs = []
                    if not last:
                        # evacuate both u PSUM banks first (cheap copies)
                        # so the next span's PV can start ~2us earlier;
                        # the normalize chain then runs off-critical
                        for h, u in ((0, u0), (1, u1)):
                            un = np_pool.tile(
                                [65, 512], F32, tag=f"un{h}", name="un"
                            )
                            nc.vector.tensor_copy(un[0:65, :], u[0:65, :])
                            srcs.append(un)
                    else:
                        # final span: nothing waits on the banks, and the
                        # copies would only delay the AllToAll gate
                        srcs = [u0, u1]
                    zbs = []
                    for h in (0, 1):
                        zinv = zp.tile([1, 512], F32, tag=f"z{h}", name="zi")
                        nc.vector.reciprocal_approx_fast(
                            zinv[0:1, :], srcs[h][0:1, :]
                        )
                        zb = zp.tile([65, 512], F32, tag=f"zb{h}", name="zb")
                        nc.gpsimd.partition_broadcast(
                            zb[0:65, :], zinv[0:1, :], channels=65
                        )
                        zbs.append(zb)
                    for h in (0, 1):
                        nrm = np_pool.tile([65, 512], BF16, tag=f"n{h}", name="nr")
                        nc.vector.tensor_tensor(
                            nrm[0:65, :],
                            srcs[h][0:65, :],
                            zbs[h][0:65, :],
                            op=ALU.mult,
                        )
                        # stripe into this span's collective buffer.
                        # batch 0: 256-col blocks, cores 2s/2s+1.
                        # batch 1: 128-col blocks, cores 4s'..4s'+3 of
                        # a2a_in1 (spans 0-1) / a2a_in2 (spans 2-3).
                        # mid-attention spans stage on the sync queue (the
                        # scalar queue is busy with exps); the last span
                        # splits across sync+scalar to halve the ~0.55us
                        # per-DMA issue latency ahead of the A2A trigger.
                        # ONE strided DMA per head covers all destination
                        # blocks: dst viewed [row-in-block, block, col],
                        # src free dim split [q-block, col] to match.
                        if b == 0:
                            dst = a2a_in0[:].rearrange(
                                "(blk r) c -> r blk c", blk=N_CORES
                            )[h * 64 : (h + 1) * 64, 2 * span : 2 * span + 2, :]
                            src = nrm[1:65, :].rearrange("r (q c) -> r q c", q=2)
                            nc.sync.dma_start(out=dst, in_=src)
                        elif span < 2:
                            # spans 0-1: 128-col blocks of a2a_in1
                            dst = a2a_in1[:].rearrange(
                                "(blk r) c -> r blk c", blk=N_CORES
                            )[h * 64 : (h + 1) * 64, 4 * span : 4 * span + 4, :]
                            src = nrm[1:65, :].rearrange("r (q c) -> r q c", q=4)
                            nc.sync.dma_start(out=dst, in_=src)
                        else:
                            # spans 2/3: own 64-col-block collectives.
                            # span 2 stages on gpsimd (SWDGE): the sync
                            # queue is clogged by the A2A#1b-gated a2a_sb
                            # loads right then. span 3 (the final trigger
                            # gate): one head per idle queue.
                            a2a_in = a2a_in1c if span == 2 else a2a_in2
                            dst = a2a_in[:].rearrange(
                                "(blk r) c -> r blk c", blk=N_CORES
                            )[h * 64 : (h + 1) * 64, :, :]
                            src = nrm[1:65, :].rearrange(
                                "r (q c) -> r q c", q=N_CORES
                            )
                            if last:
                                eng = nc.sync if h == 0 else nc.scalar
                            else:
                                eng = nc.gpsimd
                            eng.dma_start(out=dst, in_=src)

                def pv(j):
                    """exp + PV for chunk j. Emitted one iter AFTER
                    qk(j+1) and one iter BEFORE qk(j+2): the PE order per
                    iter is [prods, PV(j), QK(j+2)], so every exp's QK
                    input finished a full iteration earlier and the ACT
                    exp stream runs back-to-back (it is the ~1.17us/chunk
                    floor); the PE burst (prods+PV+QK ~1.2us) hides under
                    it. qk(j+2) reuses s(j)'s PSUM bank (ring of 2), whose
                    WAR on exp(j) resolves just before PV(j) anyway."""
                    span, ch = divmod(j, nchunks)
                    if ch == 0:
                        u_tiles[span] = (
                            up.tile([128, 512], F32, tag="u0", name="u0"),
                            up.tile([128, 512], F32, tag="u1", name="u1"),
                        )
                    u0, u1 = u_tiles[span]
                    s = s_tiles.pop(j)
                    pt = ptp.tile([128, 1024], BF16, tag="pt", name="pt")
                    gch = (b * N + ch * 128) // 128
                    # Both heads' exp on ACT. (A DVE bit-trick exp for
                    # head 1 was tried and reverted: every DVE op pays a
                    # pipeline DRAIN roughly equal to its duration, so the
                    # trick chain (~1.3us) gated PV worse than the 1.08us
                    # ACT exp it replaced.)
                    nc.scalar.activation(pt[:], s[:], AF.Exp, scale=SCALE)
                    nc.tensor.matmul(
                        u0[0:65, :],
                        vt[:, gch, 0, :],
                        pt[:, 0:512],
                        start=(ch == 0),
                        stop=(ch == nchunks - 1),
                    )
                    nc.tensor.matmul(
                        u1[0:65, :],
                        vt[:, gch, 1, :],
                        pt[:, 512:1024],
                        start=(ch == 0),
                        stop=(ch == nchunks - 1),
                    )
                    if ch == nchunks - 1:
                        finalize_span(span)

                qk(0)
                for i in range(total):
                    while pulled[0] < need[b * 64 + i]:
                        next(gen)
                        pulled[0] += 1
                    if i >= 1:
                        pv(i - 1)
                        if mid_hook is not None and (i - 1) in mid_hook:
                            mid_hook[i - 1]()
                    if i + 1 < total:
                        qk(i + 1)
                pv(total - 1)

            # ---- attention both batches; qkv/V production in the slack ----
            with (
                tc.tile_pool(name="norm", bufs=4) as np_pool,
                tc.tile_pool(name="pt", bufs=6) as ptp,
                tc.tile_pool(name="pti", bufs=4) as ptp1,
                tc.tile_pool(name="zrow", bufs=4) as zp,
                tc.tile_pool(name="vstage", bufs=2) as vsp,
                tc.tile_pool(name="s_psum", bufs=2, space="PSUM") as sp,
                tc.tile_pool(name="u_psum", bufs=1, space="PSUM") as up,
                tc.tile_pool(name="qk_psum", bufs=1, space="PSUM") as qp,
            ):
                gen = producer(qp, vsp)
                for _ in range(EAGER):
                    next(gen)
                pulled = [0]
                attention_batch(0, sp, up, np_pool, ptp, ptp1, zp, gen, pulled)

                # ---- batch 0's AllToAll, fully overlapped with batch 1's
                # attention; the a2a_sb loads for the b0 half also drain
                # here (sync queue is idle mid-attention).
                nc.gpsimd.collective_compute(
                    "AllToAll",
                    ALU.bypass,
                    replica_groups=[list(range(N_CORES))],
                    ins=[a2a_in0[:].opt()],
                    outs=[a2a_out0[:].opt()],
                )
                nc.sync.dma_start(
                    out=a2a_sb[:, :, 0 : TPC // 2],
                    in_=a2a_out0[:].rearrange("(g r) c -> r g c", g=N_CORES),
                )

                def b1_mid():
                    # batch-1 spans 0-1 staged: ship them now (overlaps
                    # the rest of batch 1), load into a2a_sb cols 256:384
                    nc.gpsimd.collective_compute(
                        "AllToAll",
                        ALU.bypass,
                        replica_groups=[list(range(N_CORES))],
                        ins=[a2a_in1[:].opt()],
                        outs=[a2a_out1[:].opt()],
                    )
                    nc.sync.dma_start(
                        out=a2a_sb[:, :, TPC // 2 : 3 * TPC // 4],
                        in_=a2a_out1[:].rearrange("(g r) c -> r g c", g=N_CORES),
                    )

                def b1_mid2():
                    # batch-1 span 2 staged: ship it too (overlaps span 3)
                    nc.gpsimd.collective_compute(
                        "AllToAll",
                        ALU.bypass,
                        replica_groups=[list(range(N_CORES))],
                        ins=[a2a_in1c[:].opt()],
                        outs=[a2a_out1c[:].opt()],
                    )
                    nc.sync.dma_start(
                        out=a2a_sb[:, :, 3 * TPC // 4 : 7 * TPC // 8],
                        in_=a2a_out1c[:].rearrange("(g r) c -> r g c", g=N_CORES),
                    )

                attention_batch(1, sp, up, np_pool, ptp, ptp1, zp, gen, pulled,
                                mid_hook={31: b1_mid, 47: b1_mid2})
                while pulled[0] < TOTAL_BG:
                    next(gen)
                    pulled[0] += 1

            if DEBUG:
                nc.sync.dma_start(out=qk_dump[:], in_=qk_sb[:])
                nc.sync.dma_start(out=vt_dump[:], in_=vt[:])

            # ---- batch-1 span 3: the only serial collective (128KB) ----
            nc.gpsimd.collective_compute(
                "AllToAll",
                ALU.bypass,
                replica_groups=[list(range(N_CORES))],
                ins=[a2a_in2[:].opt()],
                outs=[a2a_out2[:].opt()],
            )

            # ---- proj (full weights, my striped tokens), in two pieces:
            # cols 0:448 arrived with the overlapped collectives, so 7/8
            # of the proj fills the PE while the last A2A flies; only
            # cols 448:512 are serial after it.
            q3 = 7 * TPC // 8
            with (
                tc.tile_pool(name="proj_psum", bufs=4, space="PSUM") as pp,
            ):
                def proj_piece(lo, hi, tag):
                    for o in range(CCH):
                        ps = pp.tile([128, hi - lo], F32, tag=tag, name=tag)
                        for g in range(CCH):
                            nc.tensor.matmul(
                                ps[:],
                                pwt[:, g, o * 128 : (o + 1) * 128],
                                a2a_sb[:, g, lo:hi],
                                start=(g == 0),
                                stop=(g == CCH - 1),
                            )
                        nc.scalar.activation(
                            out_sb[:, o, lo:hi], ps[:], AF.Identity,
                            bias=pb[:, o : o + 1],
                        )
                        eng = nc.sync if o % 2 == 0 else nc.scalar
                        eng.dma_start(
                            out=out_d[:, o, lo:hi], in_=out_sb[:, o, lo:hi]
                        )

                proj_piece(0, q3, "pps0")
                # gpsimd (SWDGE): the sync queue is draining piece-0's
                # out DMAs when A2A#2 lands; gpsimd is idle post-trigger
                nc.gpsimd.dma_start(
                    out=a2a_sb[:, :, q3:TPC],
                    in_=a2a_out2[:].rearrange("(g r) c -> r g c", g=N_CORES),
                )
                proj_piece(q3, TPC, "pps1")

    nc.compile()
    return nc


def shard_inputs(x, qkv_w, qkv_b, proj_w, proj_b):
    """Host-side sharding + layout prep. Returns in_maps for 8 cores."""
    bf = ml_dtypes.bfloat16
    x2 = np.ascontiguousarray(x.reshape(T, C).T)  # [C, T]
    xt = np.ascontiguousarray(
        x2.reshape(CCH, 128, TB, 512).transpose(1, 2, 0, 3)
    ).astype(bf)  # [128, tb, c, 512]
    pwt_f = np.ascontiguousarray(proj_w.T)  # [j, o]
    pwt = np.ascontiguousarray(
        pwt_f.reshape(CCH, 128, C).transpose(1, 0, 2)
    ).astype(bf)
    pb = np.ascontiguousarray(proj_b.reshape(CCH, 128).T).astype(np.float32)
    ident = np.eye(128, dtype=ml_dtypes.bfloat16)

    in_maps = []
    for c in range(N_CORES):
        rows = lambda base: slice(base + 128 * c, base + 128 * (c + 1))
        wq = qkv_w[rows(0)]
        wk = qkv_w[rows(C)]
        wv = qkv_w[rows(2 * C)]
        wqkv = np.concatenate([wq, wk, wv], axis=0).T  # [C, 384]
        # j-major DRAM layout [128, 3(j), CCH, 128]
        wqkv = np.ascontiguousarray(
            wqkv.reshape(CCH, 128, 3, 128).transpose(1, 2, 0, 3)
        ).astype(bf)
        bqkv = np.stack(
            [qkv_b[rows(0)], qkv_b[rows(C)], qkv_b[rows(2 * C)]], axis=1
        ).astype(np.float32)
        in_maps.append(
            {
                "xt": xt, "wqkv": wqkv, "bqkv": bqkv, "ident": ident,
                "pwt": pwt, "pb": pb,
            }
        )
    return in_maps


_CACHED_NC = None


def kernel(x, qkv_w, qkv_b, proj_w, proj_b, _trace=False, _tmpdir=None):
    global _CACHED_NC
    x = np.asarray(x, dtype=np.float32)
    qkv_w = np.asarray(qkv_w, dtype=np.float32)
    qkv_b = np.asarray(qkv_b, dtype=np.float32)
    proj_w = np.asarray(proj_w, dtype=np.float32)
    proj_b = np.asarray(proj_b, dtype=np.float32)

    in_maps = shard_inputs(x, qkv_w, qkv_b, proj_w, proj_b)
    if _CACHED_NC is None:
        _CACHED_NC = build_bass()
    res = run_bass_kernel_spmd(
        _CACHED_NC,
        in_maps,
        core_ids=list(range(N_CORES)),
        trace=_trace,
        tmpdir=_tmpdir,
    )
    # out per core: [128, CCH, TPC] bf16; cols 0:256 = batch-0 tokens
    # [256c, 256c+256); cols 256:384 = batch-1 tokens [128c, 128c+128)
    # (spans 0-1); cols 384:448 = b1 span-2 tokens [1024+64c, +64);
    # cols 448:512 = b1 span-3 tokens [1536+64c, +64)
    out = np.empty((B, N, C), dtype=np.float32)
    for c in range(N_CORES):
        o = np.asarray(res.results[c]["out"]).astype(np.float32)
        ot = o.transpose(1, 0, 2).reshape(C, TPC)  # [1024 o, 512 t]
        out[0, c * 256 : (c + 1) * 256, :] = ot[:, 0:256].T
        out[1, c * 128 : (c + 1) * 128, :] = ot[:, 256:384].T
        out[1, 1024 + c * 64 : 1024 + (c + 1) * 64, :] = ot[:, 384:448].T
        out[1, 1536 + c * 64 : 1536 + (c + 1) * 64, :] = ot[:, 448:512].T
    if _trace:
        return out, res
    return out



# revision 39
# speedup vs baseline: 1.0032x; 1.0020x over previous
"""Distributed multi-head attention kernel for 8 TRN2 NeuronCores.

Reference computation:
    x:[2,2048,1024] -> qkv -> 16-head attention -> proj -> [2,2048,1024]

Sharding: tensor-parallel over heads (2 heads/core) for qkv + attention,
then an AllToAll switches to token sharding (512 tokens/core) for the
projection, so no AllReduce is needed and each core emits only its own
output shard.

Schedule (per core; PE computes out = lhsT.T @ rhs with contraction on
the partition axis). ACT must run one [128,1024] Exp per key-chunk
(~1.1us each, 128 chunks); the PE runs QK/PV plus all qkv production,
and under the observed power throttle (~1.2GHz effective PE clock) the
two engines are roughly balanced, so everything is pipelined:
  - x^T arrives token-block-major ([128, tb, c, 512] bf16); block 0/1
    are split across both DMA queues so span-0 q/k matmuls start right
    after the NEFF preamble; later x blocks and proj weights are DMA'd
    from inside the background stream so the sync queue stays in
    deadline order.
  - q/k computed d-major ([128 dims, 512 toks] tiles, N=512). V is also
    computed d-major (cheap N=512 matmuls, bias via tensor_scalar), then
    PE-transposed ([128,128] transpose-mode matmuls + DVE copy) into the
    per-chunk token-major [1|V] PV layout -- no DMA-xbar transposes
    (those serialize against x loads on the sync queue's semaphores).
  - attention: S^T = K.T @ Q per chunk; the two heads' QK matmuls run
    concurrently (row groups 0-63/64-127 via base_partition-derived
    tile_position), exp on ACT with the 1/8 scale folded in (no max
    subtraction: score std ~0.33), PV with lhsT=[1|V] so PSUM row 0
    accumulates the softmax denominator Z for free; the two heads' PV
    matmuls also overlap. QK runs one chunk ahead of PV.
  - all qkv/V production is yielded as background units pulled into the
    attention loop between QK(i+1) and PV(i), paced by an issue-deadline
    table (need[]): every producer instruction must be ISSUED before its
    first consumer or the Tile framework cannot order them.
  - per span: the u PSUM bank is evacuated to SBUF with one cheap DVE
    copy (so the next span's PV starts ~2us earlier), then Z row ->
    reciprocal (DVE), partition_broadcast (GPSIMD), normalize (DVE),
    DMA to the AllToAll staging buffer.
  - a tiny warm AllToAll early on absorbs the collective path's fixed
    start cost; the real AllToAll (1MB bf16) then feeds the
    token-sharded proj with full weights, bias via ACT Identity.
"""
import sys

sys.path.insert(0, "/opt/trn_rl_repo")

import numpy as np
import ml_dtypes

from concourse import bass, bacc, mybir, tile
from concourse.bass_utils import run_bass_kernel_spmd

BF16 = mybir.dt.bfloat16
F32 = mybir.dt.float32
I16 = mybir.dt.int16
AF = mybir.ActivationFunctionType
ALU = mybir.AluOpType

N_CORES = 8
B, N, C = 2, 2048, 1024
H = 16  # total heads
D = 64  # head dim
T = B * N  # 4096 flattened tokens
TPC = T // N_CORES  # tokens per core = 512
CCH = C // 128  # contraction chunks = 8
TB = T // 512  # token blocks = 8
SCALE = 1.0 / np.sqrt(D)  # 0.125
# Schraudolph bit-trick exp for head 1, bf16 variant: exp(s*SCALE) ~=
# bitcast_bf16(int16(TRICK_A*s + TRICK_B)). Scaled scores span ~+-2.2,
# far from overflow. Verified end-to-end in numpy: rel err 5.4e-3
# (gate 2e-2); halves the ACT-side exp work per chunk.
TRICK_A = (2.0 ** 7 / np.log(2.0)) * SCALE
TRICK_B = 127.0 * 128.0 - 366000.0 / 65536.0

import os as _os_mod

# PE warm-up matmuls while x DMA lands: HAM un-throttles (K=4/8 ->
# 8/8) after ~3.4us of sustained matmul activity, so burning junk
# matmuls from ~1us keeps the first real matmuls (~10us, once the
# critical DMA lands) at full clock instead of 1.2GHz.
JUNK_START = int(_os_mod.environ.get("KJUNK", "24"))
JUNK_A2A = 0  # PE keep-warm matmuls across the final AllToAll (net-negative, off)


def build_bass() -> bass.Bass:
    nc = bacc.Bacc(None, target_bir_lowering=False)

    # ---- DRAM parameters (per-core shards, host-prepared layouts) ----
    xt_d = nc.declare_dram_parameter("xt", [128, TB, CCH, 512], BF16, isOutput=False)
    # j-major (q/k/v-major) so the eager k tile's weights (j=1) land first
    wqkv_d = nc.declare_dram_parameter("wqkv", [128, 3, CCH, 128], BF16, isOutput=False)
    bqkv_d = nc.declare_dram_parameter("bqkv", [128, 3], F32, isOutput=False)
    ident_d = nc.declare_dram_parameter("ident", [128, 128], BF16, isOutput=False)
    pwt_d = nc.declare_dram_parameter("pwt", [128, CCH, C], BF16, isOutput=False)
    pb_d = nc.declare_dram_parameter("pb", [128, CCH], F32, isOutput=False)
    out_d = nc.declare_dram_parameter("out", [128, CCH, TPC], BF16, isOutput=True)
    import os
    DEBUG = os.environ.get("KDEBUG", "0") == "1"
    if DEBUG:
        qk_dump = nc.declare_dram_parameter("qk_dump", [128, 2, T], BF16, isOutput=True)
        vt_dump = nc.declare_dram_parameter("vt_dump", [128, T // 128, 2, 65], BF16, isOutput=True)

    with tile.TileContext(nc) as tc:
        with (
            tc.tile_pool(name="const", bufs=1) as const,
            tc.tile_pool(name="weights", bufs=1) as wpool,
            tc.tile_pool(name="acts", bufs=1) as apool,
            tc.tile_pool(name="dram", bufs=1, space="DRAM") as dram,
        ):
            # ---- resident SBUF tensors ----
            xt = wpool.tile([128, TB, CCH, 512], BF16)
            wqkv = wpool.tile([128, CCH, 384], BF16)
            pwt = wpool.tile([128, CCH, C], BF16)
            pb = const.tile([128, CCH], F32)
            bqkv = const.tile([128, 3], F32)
            ident = const.tile([128, 128], BF16)

            qk_sb = apool.tile([128, 2, T], BF16)  # j-major q/k
            # PV stationary: per t2-chunk, per head: [1 | V_h]
            vt = apool.tile([128, T // 128, 2, 65], BF16)
            a2a_sb = apool.tile([128, CCH, TPC], BF16)
            out_sb = apool.tile([128, CCH, TPC], BF16)
            jwarm = const.tile([128, 512], BF16)
            wsrc = const.tile([128, 32], F32)
            wdst = const.tile([128, 32], BF16)

            # token-striped AllToAll in 3 pieces: core c owns 256 tokens
            # of batch 0 (shipped after batch 0, fully overlapped), 128
            # tokens of batch 1 spans 0-1 (shipped mid-batch-1, also
            # overlapped), and 128 tokens of spans 2-3 (the only serial
            # collective, 256KB). The tail then runs 3/4 of the proj
            # while the last A2A flies.
            a2a_in0 = dram.tile([N_CORES * 128, TPC // 2], BF16)
            a2a_out0 = dram.tile([N_CORES * 128, TPC // 2], BF16)
            a2a_in1 = dram.tile([N_CORES * 128, TPC // 4], BF16)
            a2a_out1 = dram.tile([N_CORES * 128, TPC // 4], BF16)
            a2a_in1c = dram.tile([N_CORES * 128, TPC // 8], BF16)
            a2a_out1c = dram.tile([N_CORES * 128, TPC // 8], BF16)
            a2a_in2 = dram.tile([N_CORES * 128, TPC // 8], BF16)
            a2a_out2 = dram.tile([N_CORES * 128, TPC // 8], BF16)
            warm_in = dram.tile([N_CORES, 16], BF16)
            warm_out = dram.tile([N_CORES, 16], BF16)

            # ---- load inputs. sync queue: weights + most of x; scalar
            # queue: small consts + the other half of tb0/tb1 so span-0
            # inputs land fast. pwt/pb late (needed only at proj).
            # critical path first: the eager k0 tile's c=0 matmul needs
            # only wqkv[0:2] + xt[b0,c0:2] (~0.45MB), so those two land
            # before the bulk and the PE starts ~5us earlier.
            # strict priority order on ONE queue: the early phase is DMA-
            # bandwidth-bound (~0.2-0.4 MB/us), so a second queue's bulk
            # transfer steals bandwidth from the critical path. Eager
            # order of consumption: k0 (wqkv j=1 + xt b0), q0 (j=0),
            # v0 (j=2), then k1/v1 (xt b1).
            nc.sync.dma_start(out=wqkv[:, :, 128:256], in_=wqkv_d[:, 1])
            nc.sync.dma_start(out=xt[:, 0, 0:3, :], in_=xt_d[:, 0, 0:3, :])
            nc.sync.dma_start(out=wqkv[:, :, 0:128], in_=wqkv_d[:, 0])
            nc.sync.dma_start(out=xt[:, 0, 3:6, :], in_=xt_d[:, 0, 3:6, :])
            nc.sync.dma_start(out=wqkv[:, :, 256:384], in_=wqkv_d[:, 2])
            nc.sync.dma_start(out=xt[:, 0, 6:8, :], in_=xt_d[:, 0, 6:8, :])
            nc.scalar.dma_start(out=bqkv[:], in_=bqkv_d[:])
            nc.scalar.dma_start(out=ident[:], in_=ident_d[:])
            nc.sync.dma_start(out=xt[:, 1, 0:4, :], in_=xt_d[:, 1, 0:4, :])
            nc.sync.dma_start(out=xt[:, 1, 4:8, :], in_=xt_d[:, 1, 4:8, :])
            nc.sync.dma_start(out=xt[:, 2, :, :], in_=xt_d[:, 2, :, :])
            nc.sync.dma_start(out=xt[:, 3, :, :], in_=xt_d[:, 3, :, :])

            # preload the Exp table set so span 0 doesn't pay ~2.7us
            nc.vector.memset(wsrc[:], 0.0)
            nc.scalar.activation(wdst[:], wsrc[:], AF.Exp, scale=1.0)
            nc.vector.memset(jwarm[:], 0.25)
            nc.vector.memset(vt[:, :, :, 0:1], 1.0)  # Z ones columns

            # tiny dummy AllToAll: warms the collective path so the real
            # A2A's fixed start latency overlaps compute instead. Optional:
            # its completion semaphore gates later sync-queue DMA setup
            # (semaphore reuse), which can stall the V transposes.
            import os as _os
            if _os.environ.get("KWARM_A2A", "1") == "1":
                nc.gpsimd.collective_compute(
                    "AllToAll",
                    ALU.bypass,
                    replica_groups=[list(range(N_CORES))],
                    ins=[warm_in[:].opt()],
                    outs=[warm_out[:].opt()],
                )

            # ---- PE warm-up: HAM flips to 2.4GHz after ~3.4us of
            # sustained matmul activity; burn that in while x DMA lands.
            with tc.tile_pool(name="jw_psum", bufs=2, space="PSUM") as jwp:
                for _ in range(JUNK_START):
                    jt = jwp.tile([128, 512], F32, tag="jk", name="jk")
                    nc.tensor.matmul(
                        jt[:],
                        jwarm[:, 0:128],
                        jwarm[:],
                        start=True,
                        stop=True,
                        skip_group_check=True,
                    )

            # ---- background qkv/V production (pulled into PE slack) ----
            def producer(qp, vsp):
                def qk_tile(j, tb):
                    """One [128 dims, 512 toks] q (j=0) or k (j=1) tile."""
                    ps = qp.tile([128, 512], F32, tag="qkt", name="qkt")
                    for c in range(CCH):
                        nc.tensor.matmul(
                            ps[:],
                            wqkv[:, c, j * 128 : (j + 1) * 128],
                            xt[:, tb, c, :],
                            start=(c == 0),
                            stop=(c == CCH - 1),
                            skip_group_check=True,
                        )
                        yield
                    nc.vector.tensor_scalar_add(
                        qk_sb[:, j, tb * 512 : (tb + 1) * 512],
                        ps[:],
                        bqkv[:, j : j + 1],
                    )
                    yield

                def v_tile(tb):
                    """V d-major tile [128 vdims, 512 toks] (N=512 matmuls,
                    cheap), bias folded in d-major, then 4 PE transposes
                    into the per-chunk token-major [1|V] PV layout."""
                    ps = qp.tile([128, 512], F32, tag="qkt", name="vtl")
                    for c in range(CCH):
                        nc.tensor.matmul(
                            ps[:],
                            wqkv[:, c, 256:384],
                            xt[:, tb, c, :],
                            start=(c == 0),
                            stop=(c == CCH - 1),
                            skip_group_check=True,
                        )
                        yield
                    vs = vsp.tile([128, 512], BF16, tag="vst", name="vst")
                    nc.vector.tensor_scalar_add(vs[:], ps[:], bqkv[:, 2:3])
                    yield
                    for u in range(4):
                        ch = 4 * tb + u
                        # single-buffered: PSUM tags are bank-rounded
                        # and all 8 banks are allocated, so the transpose
                        # cannot be double-buffered
                        tt = qp.tile([128, 128], BF16, tag="ttl", name="ttl")
                        nc.tensor.transpose(
                            tt[:], vs[:, u * 128 : (u + 1) * 128], ident[:]
                        )
                        yield
                        nc.vector.tensor_copy(vt[:, ch, :, 1:65], tt[:, :])
                        yield

                def x_load(tb):
                    """Deferred input DMA issue, ordered into the sync
                    queue between the generator's transposes."""
                    if tb < TB:
                        nc.sync.dma_start(
                            out=xt[:, tb, :, :], in_=xt_d[:, tb, :, :]
                        )
                    elif tb == TB:
                        nc.sync.dma_start(out=pwt[:], in_=pwt_d[:])
                    else:
                        nc.sync.dma_start(out=pb[:], in_=pb_d[:])
                    yield

                # eager prefix: span-0 needs k0, q0 and the first V chunks
                yield from qk_tile(1, 0)
                yield from qk_tile(0, 0)
                yield from v_tile(0)
                # background, deadline order (see BG_ORDER)
                for kind, idx, _d, _n in BG_ORDER:
                    if kind == "k":
                        yield from qk_tile(1, idx)
                    elif kind == "q":
                        yield from qk_tile(0, idx)
                    elif kind == "x":
                        yield from x_load(idx)
                    else:
                        yield from v_tile(idx)
                while True:
                    yield

            QKU = CCH + 1  # yields per q/k tile
            VU = CCH + 1 + 8  # yields per v tile (8 mm, bias, 4x(tp+copy))
            EAGER = 2 * QKU + VU  # k0, q0, v0

            # Background units: (kind, idx, issue-deadline, yields).
            # deadline = global chunk index by whose END the unit must be
            # fully ISSUED -- program order must put every producer before
            # its first consumer or Tile cannot see the dependency.
            # k(tb): first read by qk(4*(tb%4)) of its batch, one chunk
            # early. q(tb): first read by qk(16*s). v(tb): chunks
            # 4tb..4tb+3, first read by PV(4*(tb%4)). x(tb): deferred
            # input DMA issue; x8/x9 = pwt/pb.
            BG_ORDER = [
                ("k", 1, 1, QKU), ("v", 1, 2, VU), ("k", 2, 5, QKU),
                ("v", 2, 6, VU), ("k", 3, 9, QKU), ("v", 3, 10, VU),
                ("q", 1, 13, QKU), ("q", 2, 20, QKU), ("x", 4, 22, 1),
                ("x", 5, 26, 1), ("q", 3, 28, QKU), ("x", 6, 32, 1),
                ("x", 7, 34, 1), ("q", 4, 38, QKU), ("k", 4, 42, QKU),
                ("x", 8, 44, 1), ("x", 9, 45, 1), ("v", 4, 58, VU),
                ("k", 5, 65, QKU), ("v", 5, 66, VU), ("k", 6, 69, QKU),
                ("v", 6, 70, VU), ("k", 7, 73, QKU), ("v", 7, 74, VU),
                ("q", 5, 77, QKU), ("q", 6, 93, QKU), ("q", 7, 109, QKU),
            ]
            TOTAL_BG = sum(n for _, _, _, n in BG_ORDER)
            # need[g] = total bg yields that must be issued by the end of
            # global chunk g
            need = [0] * 128
            for _kind, _idx, _d, _n in BG_ORDER:
                for _g in range(max(_d, 0), 128):
                    need[_g] += _n

            def attention_batch(b, sp, up, np_pool, ptp, ptp1, zp, gen, pulled,
                                mid_hook=None):
                """Attention for batch b; flat chunk loop, QK one chunk
                ahead of PV across span boundaries."""
                nchunks = N // 128
                nspans = N // 512
                total = nspans * nchunks
                s_tiles = {}
                u_tiles = {}

                def qk(i):
                    span, ch = divmod(i, nchunks)
                    t1 = b * N + span * 512
                    t2 = b * N + ch * 128
                    s = sp.tile([128, 1024], F32, tag="s", name="s")
                    s_tiles[i] = s
                    nc.tensor.matmul(
                        s[:, 0:512],
                        qk_sb[0:64, 1, t2 : t2 + 128],
                        qk_sb[0:64, 0, t1 : t1 + 512],
                        start=True,
                        stop=True,
                    )
                    nc.tensor.matmul(
                        s[:, 512:1024],
                        qk_sb[64:128, 1, t2 : t2 + 128],
                        qk_sb[64:128, 0, t1 : t1 + 512],
                        start=True,
                        stop=True,
                    )

                def finalize_span(span):
                    u0, u1 = u_tiles.pop(span)
                    last = b == 1 and span == nspans - 1
                    srcs = []
                    if not last:
                        # evacuate both u PSUM banks first (cheap copies)
                        # so the next span's PV can start ~2us earlier;
                        # the normalize chain then runs off-critical
                        for h, u in ((0, u0), (1, u1)):
                            un = np_pool.tile(
                                [65, 512], F32, tag=f"un{h}", name="un"
                            )
                            nc.vector.tensor_copy(un[0:65, :], u[0:65, :])
                            srcs.append(un)
                    else:
                        # final span: nothing waits on the banks, and the
                        # copies would only delay the AllToAll gate
                        srcs = [u0, u1]
                    zbs = []
                    for h in (0, 1):
                        zinv = zp.tile([1, 512], F32, tag=f"z{h}", name="zi")
                        nc.vector.reciprocal_approx_fast(
                            zinv[0:1, :], srcs[h][0:1, :]
                        )
                        zb = zp.tile([65, 512], F32, tag=f"zb{h}", name="zb")
                        nc.gpsimd.partition_broadcast(
                            zb[0:65, :], zinv[0:1, :], channels=65
                        )
                        zbs.append(zb)
                    for h in (0, 1):
                        nrm = np_pool.tile([65, 512], BF16, tag=f"n{h}", name="nr")
                        nc.vector.tensor_tensor(
                            nrm[0:65, :],
                            srcs[h][0:65, :],
                            zbs[h][0:65, :],
                            op=ALU.mult,
                        )
                        # stripe into this span's collective buffer.
                        # batch 0: 256-col blocks, cores 2s/2s+1.
                        # batch 1: 128-col blocks, cores 4s'..4s'+3 of
                        # a2a_in1 (spans 0-1) / a2a_in2 (spans 2-3).
                        # mid-attention spans stage on the sync queue (the
                        # scalar queue is busy with exps); the last span
                        # splits across sync+scalar to halve the ~0.55us
                        # per-DMA issue latency ahead of the A2A trigger.
                        # ONE strided DMA per head covers all destination
                        # blocks: dst viewed [row-in-block, block, col],
                        # src free dim split [q-block, col] to match.
                        if b == 0:
                            dst = a2a_in0[:].rearrange(
                                "(blk r) c -> r blk c", blk=N_CORES
                            )[h * 64 : (h + 1) * 64, 2 * span : 2 * span + 2, :]
                            src = nrm[1:65, :].rearrange("r (q c) -> r q c", q=2)
                            nc.sync.dma_start(out=dst, in_=src)
                        elif span < 2:
                            # spans 0-1: 128-col blocks of a2a_in1
                            dst = a2a_in1[:].rearrange(
                                "(blk r) c -> r blk c", blk=N_CORES
                            )[h * 64 : (h + 1) * 64, 4 * span : 4 * span + 4, :]
                            src = nrm[1:65, :].rearrange("r (q c) -> r q c", q=4)
                            nc.sync.dma_start(out=dst, in_=src)
                        else:
                            # spans 2/3: own 64-col-block collectives.
                            # span 2 stages on gpsimd (SWDGE): the sync
                            # queue is clogged by the A2A#1b-gated a2a_sb
                            # loads right then. span 3 (the final trigger
                            # gate): one head per idle queue.
                            a2a_in = a2a_in1c if span == 2 else a2a_in2
                            dst = a2a_in[:].rearrange(
                                "(blk r) c -> r blk c", blk=N_CORES
                            )[h * 64 : (h + 1) * 64, :, :]
                            src = nrm[1:65, :].rearrange(
                                "r (q c) -> r q c", q=N_CORES
                            )
                            if last:
                                eng = nc.sync if h == 0 else nc.scalar
                            else:
                                eng = nc.gpsimd
                            eng.dma_start(out=dst, in_=src)

                def pv(j):
                    """exp + PV for chunk j. Emitted one iter AFTER
                    qk(j+1) and one iter BEFORE qk(j+2): the PE order per
                    iter is [prods, PV(j), QK(j+2)], so every exp's QK
                    input finished a full iteration earlier and the ACT
                    exp stream runs back-to-back (it is the ~1.17us/chunk
                    floor); the PE burst (prods+PV+QK ~1.2us) hides under
                    it. qk(j+2) reuses s(j)'s PSUM bank (ring of 2), whose
                    WAR on exp(j) resolves just before PV(j) anyway."""
                    span, ch = divmod(j, nchunks)
                    if ch == 0:
                        u_tiles[span] = (
                            up.tile([128, 512], F32, tag="u0", name="u0"),
                            up.tile([128, 512], F32, tag="u1", name="u1"),
                        )
                    u0, u1 = u_tiles[span]
                    s = s_tiles.pop(j)
                    pt = ptp.tile([128, 1024], BF16, tag="pt", name="pt")
                    gch = (b * N + ch * 128) // 128
                    # Both heads' exp on ACT. (A DVE bit-trick exp for
                    # head 1 was tried and reverted: every DVE op pays a
                    # pipeline DRAIN roughly equal to its duration, so the
                    # trick chain (~1.3us) gated PV worse than the 1.08us
                    # ACT exp it replaced.)
                    nc.scalar.activation(pt[:], s[:], AF.Exp, scale=SCALE)
                    nc.tensor.matmul(
                        u0[0:65, :],
                        vt[:, gch, 0, :],
                        pt[:, 0:512],
                        start=(ch == 0),
                        stop=(ch == nchunks - 1),
                    )
                    nc.tensor.matmul(
                        u1[0:65, :],
                        vt[:, gch, 1, :],
                        pt[:, 512:1024],
                        start=(ch == 0),
                        stop=(ch == nchunks - 1),
                    )
                    if ch == nchunks - 1:
                        finalize_span(span)

                qk(0)
                for i in range(total):
                    while pulled[0] < need[b * 64 + i]:
                        next(gen)
                        pulled[0] += 1
                    if i >= 1:
                        pv(i - 1)
                        if mid_hook is not None and (i - 1) in mid_hook:
                            mid_hook[i - 1]()
                    if i + 1 < total:
                        qk(i + 1)
                pv(total - 1)

            # ---- attention both batches; qkv/V production in the slack ----
            with (
                tc.tile_pool(name="norm", bufs=4) as np_pool,
                tc.tile_pool(name="pt", bufs=6) as ptp,
                tc.tile_pool(name="pti", bufs=4) as ptp1,
                tc.tile_pool(name="zrow", bufs=4) as zp,
                tc.tile_pool(name="vstage", bufs=2) as vsp,
                tc.tile_pool(name="s_psum", bufs=2, space="PSUM") as sp,
                tc.tile_pool(name="u_psum", bufs=1, space="PSUM") as up,
                tc.tile_pool(name="qk_psum", bufs=1, space="PSUM") as qp,
            ):
                gen = producer(qp, vsp)
                for _ in range(EAGER):
                    next(gen)
                pulled = [0]
                attention_batch(0, sp, up, np_pool, ptp, ptp1, zp, gen, pulled)

                # ---- batch 0's AllToAll, fully overlapped with batch 1's
                # attention; the a2a_sb loads for the b0 half also drain
                # here (sync queue is idle mid-attention).
                nc.gpsimd.collective_compute(
                    "AllToAll",
                    ALU.bypass,
                    replica_groups=[list(range(N_CORES))],
                    ins=[a2a_in0[:].opt()],
                    outs=[a2a_out0[:].opt()],
                )
                nc.sync.dma_start(
                    out=a2a_sb[:, :, 0 : TPC // 2],
                    in_=a2a_out0[:].rearrange("(g r) c -> r g c", g=N_CORES),
                )

                def b1_mid():
                    # batch-1 spans 0-1 staged: ship them now (overlaps
                    # the rest of batch 1), load into a2a_sb cols 256:384
                    nc.gpsimd.collective_compute(
                        "AllToAll",
                        ALU.bypass,
                        replica_groups=[list(range(N_CORES))],
                        ins=[a2a_in1[:].opt()],
                        outs=[a2a_out1[:].opt()],
                    )
                    nc.sync.dma_start(
                        out=a2a_sb[:, :, TPC // 2 : 3 * TPC // 4],
                        in_=a2a_out1[:].rearrange("(g r) c -> r g c", g=N_CORES),
                    )

                def b1_mid2():
                    # batch-1 span 2 staged: ship it too (overlaps span 3)
                    nc.gpsimd.collective_compute(
                        "AllToAll",
                        ALU.bypass,
                        replica_groups=[list(range(N_CORES))],
                        ins=[a2a_in1c[:].opt()],
                        outs=[a2a_out1c[:].opt()],
                    )
                    nc.sync.dma_start(
                        out=a2a_sb[:, :, 3 * TPC // 4 : 7 * TPC // 8],
                        in_=a2a_out1c[:].rearrange("(g r) c -> r g c", g=N_CORES),
                    )

                attention_batch(1, sp, up, np_pool, ptp, ptp1, zp, gen, pulled,
                                mid_hook={31: b1_mid, 47: b1_mid2})
                while pulled[0] < TOTAL_BG:
                    next(gen)
                    pulled[0] += 1

            if DEBUG:
                nc.sync.dma_start(out=qk_dump[:], in_=qk_sb[:])
                nc.sync.dma_start(out=vt_dump[:], in_=vt[:])

            # ---- batch-1 span 3: the only serial collective (128KB) ----
            nc.gpsimd.collective_compute(
                "AllToAll",
                ALU.bypass,
                replica_groups=[list(range(N_CORES))],
                ins=[a2a_in2[:].opt()],
                outs=[a2a_out2[:].opt()],
            )

            # ---- proj (full weights, my striped tokens), in three
            # pieces so each gates only on its own collective: cols 0:384
            # (b0 + b1 spans 0-1, ready well before attention ends),
            # 384:448 (span 2's A2A, usually done mid-attention but late
            # on high-skew runs), 448:512 (the final A2A).
            q2 = 3 * TPC // 4
            q3 = 7 * TPC // 8
            with (
                tc.tile_pool(name="proj_psum", bufs=2, space="PSUM") as pp,
            ):
                def proj_piece(lo, hi, tag):
                    for o in range(CCH):
                        ps = pp.tile([128, hi - lo], F32, tag=tag, name=tag)
                        for g in range(CCH):
                            nc.tensor.matmul(
                                ps[:],
                                pwt[:, g, o * 128 : (o + 1) * 128],
                                a2a_sb[:, g, lo:hi],
                                start=(g == 0),
                                stop=(g == CCH - 1),
                            )
                        nc.scalar.activation(
                            out_sb[:, o, lo:hi], ps[:], AF.Identity,
                            bias=pb[:, o : o + 1],
                        )
                        eng = nc.sync if o % 2 == 0 else nc.scalar
                        eng.dma_start(
                            out=out_d[:, o, lo:hi], in_=out_sb[:, o, lo:hi]
                        )

                proj_piece(0, q2, "pps0")
                proj_piece(q2, q3, "pps1")
                # gpsimd (SWDGE): the sync queue is draining piece-0's
                # out DMAs when A2A#2 lands; gpsimd is idle post-trigger
                nc.gpsimd.dma_start(
                    out=a2a_sb[:, :, q3:TPC],
                    in_=a2a_out2[:].rearrange("(g r) c -> r g c", g=N_CORES),
                )
                proj_piece(q3, TPC, "pps2")

    nc.compile()
    return nc


def shard_inputs(x, qkv_w, qkv_b, proj_w, proj_b):
    """Host-side sharding + layout prep. Returns in_maps for 8 cores."""
    bf = ml_dtypes.bfloat16
    x2 = np.ascontiguousarray(x.reshape(T, C).T)  # [C, T]
    xt = np.ascontiguousarray(
        x2.reshape(CCH, 128, TB, 512).transpose(1, 2, 0, 3)
    ).astype(bf)  # [128, tb, c, 512]
    pwt_f = np.ascontiguousarray(proj_w.T)  # [j, o]
    pwt = np.ascontiguousarray(
        pwt_f.reshape(CCH, 128, C).transpose(1, 0, 2)
    ).astype(bf)
    pb = np.ascontiguousarray(proj_b.reshape(CCH, 128).T).astype(np.float32)
    ident = np.eye(128, dtype=ml_dtypes.bfloat16)

    in_maps = []
    for c in range(N_CORES):
        rows = lambda base: slice(base + 128 * c, base + 128 * (c + 1))
        wq = qkv_w[rows(0)]
        wk = qkv_w[rows(C)]
        wv = qkv_w[rows(2 * C)]
        wqkv = np.concatenate([wq, wk, wv], axis=0).T  # [C, 384]
        # j-major DRAM layout [128, 3(j), CCH, 128]
        wqkv = np.ascontiguousarray(
            wqkv.reshape(CCH, 128, 3, 128).transpose(1, 2, 0, 3)
        ).astype(bf)
        bqkv = np.stack(
            [qkv_b[rows(0)], qkv_b[rows(C)], qkv_b[rows(2 * C)]], axis=1
        ).astype(np.float32)
        in_maps.append(
            {
                "xt": xt, "wqkv": wqkv, "bqkv": bqkv, "ident": ident,
                "pwt": pwt, "pb": pb,
            }
        )
    return in_maps


_CACHED_NC = None


def kernel(x, qkv_w, qkv_b, proj_w, proj_b, _trace=False, _tmpdir=None):
    global _CACHED_NC
    x = np.asarray(x, dtype=np.float32)
    qkv_w = np.asarray(qkv_w, dtype=np.float32)
    qkv_b = np.asarray(qkv_b, dtype=np.float32)
    proj_w = np.asarray(proj_w, dtype=np.float32)
    proj_b = np.asarray(proj_b, dtype=np.float32)

    in_maps = shard_inputs(x, qkv_w, qkv_b, proj_w, proj_b)
    if _CACHED_NC is None:
        _CACHED_NC = build_bass()
    res = run_bass_kernel_spmd(
        _CACHED_NC,
        in_maps,
        core_ids=list(range(N_CORES)),
        trace=_trace,
        tmpdir=_tmpdir,
    )
    # out per core: [128, CCH, TPC] bf16; cols 0:256 = batch-0 tokens
    # [256c, 256c+256); cols 256:384 = batch-1 tokens [128c, 128c+128)
    # (spans 0-1); cols 384:448 = b1 span-2 tokens [1024+64c, +64);
    # cols 448:512 = b1 span-3 tokens [1536+64c, +64)
    out = np.empty((B, N, C), dtype=np.float32)
    for c in range(N_CORES):
        o = np.asarray(res.results[c]["out"]).astype(np.float32)
        ot = o.transpose(1, 0, 2).reshape(C, TPC)  # [1024 o, 512 t]
        out[0, c * 256 : (c + 1) * 256, :] = ot[:, 0:256].T
        out[1, c * 128 : (c + 1) * 128, :] = ot[:, 256:384].T
        out[1, 1024 + c * 64 : 1024 + (c + 1) * 64, :] = ot[:, 384:448].T
        out[1, 1536 + c * 64 : 1536 + (c + 1) * 64, :] = ot[:, 448:512].T
    if _trace:
        return out, res
    return out

